# revision 18
# baseline (speedup 1.0000x reference)
"""Trainium2 Bass kernel for a DGL-style InteractionNetwork (GNN message passing).

Strategy v4 (edge-parallel, zero collectives, zero device-side gather):
  * Host permutes nodes into 1600 balanced 64-node blocks (LPT bin-packing on
    receiver degree) so every block owns <= S*128 edges with S minimal (5).
    200 blocks per core; the per-core segment-sum is core-local.
  * Host gathers sender/receiver node features into edge-slot order, so the
    device sees dense bf16 streams and never does an indirect DMA:
      stream1[:, e] = [ef_e | nf[send_e]]        (128 rows)
      stream2[:, e] = [nf[recv_e] | 1]           (65 rows; 0 for pad slots)
  * Device, per 128-edge slice: two PSUM-accumulated matmuls (K=128, K=65)
    give the edge-MLP hidden pre-activation; one-hot (DVE is_equal) segment
    sum of the hidden layer on the PE.  4 slices share a PSUM bank so the
    relu is a single wide ACT op; aggregation matmuls trail one group so the
    PE never waits on the relu.
  * 8 blocks form a 512-node window in one PSUM bank; when a window
    completes, its node-MLP chunk (bf16, We2 folded in) is issued inline,
    two groups later, so phase C fully overlaps phase B.
  * Host scatters per-core outputs back through the node permutation.
"""

import numpy as np
import ml_dtypes

BF = ml_dtypes.bfloat16

N_NODES = 100000
N_EDGES = 1000000
D = 64
HID = 128
CORES = 8
BLK = 64                           # nodes per one-hot block
NBLK = 200                         # blocks per core
NBLK_ALL = NBLK * CORES            # 1600
NLOC_PAD = NBLK * BLK              # 12800 node slots per core
WIN = 8                            # blocks per PSUM aggregation window
CH = WIN * BLK                     # 512 nodes per phase-C chunk
PAD_RB = 200.0                     # one-hot-miss sentinel for pad edges

_prog_cache = {}


def _build(S):
    import concourse.mybir as mybir
    import concourse.tile as tile
    from concourse import bacc

    bf16 = mybir.dt.bfloat16
    f32 = mybir.dt.float32
    Relu = mybir.ActivationFunctionType.Relu
    Ident = mybir.ActivationFunctionType.Identity
    EQ = mybir.AluOpType.is_equal

    T = NBLK * S                   # total 128-edge slices per core
    SB = 4                         # blocks per stream-DMA chunk
    CW = SB * S * 128              # stream columns per chunk
    NCH = SB * S                   # slices per chunk
    GR = 8                         # slices per wide-relu group (2 PSUM banks)
    assert T % GR == 0 and T % NCH == 0 and NBLK % WIN == 0

    nc = bacc.Bacc("TRN2", target_bir_lowering=False, debug=False,
                   num_devices=CORES)

    s1_d = nc.dram_tensor("s1", [128, T * 128], bf16, kind="ExternalInput")
    s2_d = nc.dram_tensor("s2", [65, T * 128], bf16, kind="ExternalInput")
    rb_d = nc.dram_tensor("rb", [128, T], f32, kind="ExternalInput")
    nfloc_d = nc.dram_tensor("nfloc", [64, NLOC_PAD], bf16, kind="ExternalInput")
    deg_d = nc.dram_tensor("deg", [1, NLOC_PAD], bf16, kind="ExternalInput")
    wa_d = nc.dram_tensor("wa", [128, HID], bf16, kind="ExternalInput")
    wb_d = nc.dram_tensor("wb", [65, HID], bf16, kind="ExternalInput")
    wh1_d = nc.dram_tensor("wh1", [HID, HID], bf16, kind="ExternalInput")
    wn1n_d = nc.dram_tensor("wn1n", [64, HID], bf16, kind="ExternalInput")
    c1_d = nc.dram_tensor("c1", [1, HID], bf16, kind="ExternalInput")
    bn1_d = nc.dram_tensor("bn1c", [HID, 1], f32, kind="ExternalInput")
    wn2_d = nc.dram_tensor("wn2", [HID, D], bf16, kind="ExternalInput")
    bn2_d = nc.dram_tensor("bn2c", [D, 1], f32, kind="ExternalInput")
    iota_d = nc.dram_tensor("iota", [128, BLK], bf16, kind="ExternalInput")
    out_d = nc.dram_tensor("out_t", [64, NLOC_PAD], bf16, kind="ExternalOutput")

    with tile.TileContext(nc) as tc:
        with tc.tile_pool(name="const", bufs=1) as cp, \
             tc.tile_pool(name="s1p", bufs=4) as s1p, \
             tc.tile_pool(name="s2p", bufs=4) as s2p, \
             tc.tile_pool(name="hw", bufs=3) as hwp, \
             tc.tile_pool(name="wrk", bufs=4) as wp:

            def cload(d, shape, dtype, tag, eng=None):
                t = cp.tile(shape, dtype, tag=tag)
                (eng or nc.sync).dma_start(t[:], d[:])
                return t

            # hot constants first on the SP ring so the first stream chunks
            # are not queued behind the big phase-C loads (those go on the
            # ACT ring instead).
            wa = cload(wa_d, [128, HID], bf16, "wa")
            wb = cload(wb_d, [65, HID], bf16, "wb")
            iota = cload(iota_d, [128, BLK], bf16, "iota")

            chunk_tiles = {}

            def issue_chunk(tstart):
                s1c = s1p.tile([128, CW], bf16, tag="s1c", name="s1c")
                nc.sync.dma_start(s1c[:], s1_d[:, tstart * 128:
                                               (tstart + NCH) * 128])
                s2c = s2p.tile([65, CW], bf16, tag="s2c", name="s2c")
                nc.gpsimd.dma_start(s2c[:], s2_d[:, tstart * 128:
                                                 (tstart + NCH) * 128])
                chunk_tiles[tstart] = (s1c, s2c)

            issue_chunk(0)
            issue_chunk(NCH)
            rball = cload(rb_d, [128, T], f32, "rball")
            wh1 = cload(wh1_d, [HID, HID], bf16, "wh1", nc.scalar)
            wn1n = cload(wn1n_d, [64, HID], bf16, "wn1n", nc.scalar)
            c1 = cload(c1_d, [1, HID], bf16, "c1", nc.scalar)
            bn1 = cload(bn1_d, [HID, 1], f32, "bn1", nc.scalar)
            wn2 = cload(wn2_d, [HID, D], bf16, "wn2", nc.scalar)
            bn2 = cload(bn2_d, [D, 1], f32, "bn2", nc.scalar)
            nfloc = cload(nfloc_d, [64, NLOC_PAD], bf16, "nfloc", nc.scalar)
            degall = cload(deg_d, [1, NLOC_PAD], bf16, "degall", nc.scalar)

            with tc.tile_pool(name="psB", bufs=2, space="PSUM") as psB, \
                 tc.tile_pool(name="psA", bufs=2, space="PSUM") as psA, \
                 tc.tile_pool(name="psC", bufs=1, space="PSUM") as psC, \
                 tc.tile_pool(name="psCo", bufs=1, space="PSUM") as psCo, \
                 tc.tile_pool(name="ohp", bufs=2 * GR + 2) as ohp, \
                 tc.tile_pool(name="hidp", bufs=3) as hidp:

                pend = []          # slices whose agg matmul is not yet issued
                pend_c = []        # completed windows awaiting phase C
                agg_tile = [None]

                def phase_c(w, haggw):
                    n0 = w * CH
                    p1 = psC.tile([HID, CH], f32, tag="p1", name="p1")
                    nc.tensor.matmul(out=p1[:], lhsT=wh1[:], rhs=haggw[:],
                                     start=True, stop=False)
                    nc.tensor.matmul(out=p1[:], lhsT=wn1n[:],
                                     rhs=nfloc[:, n0:n0 + CH],
                                     start=False, stop=False)
                    nc.tensor.matmul(out=p1[:], lhsT=c1[:],
                                     rhs=degall[:, n0:n0 + CH],
                                     start=False, stop=True)
                    nh = wp.tile([HID, CH], bf16, tag="nh", name="nh")
                    nc.scalar.activation(out=nh[:], in_=p1[:],
                                         func=Relu, bias=bn1[:, 0:1])
                    po = psCo.tile([D, CH], f32, tag="po", name="po")
                    nc.tensor.matmul(out=po[:], lhsT=wn2[:], rhs=nh[:],
                                     start=True, stop=True)
                    oc = wp.tile([D, CH], bf16, tag="oc", name="oc")
                    nc.scalar.activation(out=oc[:], in_=po[:],
                                         func=Ident, bias=bn2[:, 0:1])
                    nc.sync.dma_start(out_d[:, n0:n0 + CH], oc[:])

                def issue_agg(hidw_, r_, oh_, b_, s_):
                    wb_ = b_ % WIN
                    if wb_ == 0 and s_ == 0:
                        agg_tile[0] = psA.tile([HID, WIN * BLK], f32,
                                               tag="paw", name="paw")
                    pa = agg_tile[0]
                    nc.tensor.matmul(out=pa[:, wb_ * BLK:(wb_ + 1) * BLK],
                                     lhsT=hidw_[:, r_ * 128:(r_ + 1) * 128],
                                     rhs=oh_[:], start=(s_ == 0),
                                     stop=(s_ == S - 1))
                    if wb_ == WIN - 1 and s_ == S - 1:
                        haggw = hwp.tile([HID, WIN * BLK], bf16, tag="hagw",
                                         name="hagw")
                        nc.vector.tensor_copy(out=haggw[:], in_=pa[:])
                        pend_c.append([b_ // WIN, haggw, 3])

                s1c = s2c = None
                for g in range(T // GR):
                    # phase-C chunks issue two groups after their window ends
                    for item in pend_c:
                        item[2] -= 1
                    while pend_c and pend_c[0][2] <= 0:
                        w_, hg_, _ = pend_c.pop(0)
                        phase_c(w_, hg_)
                    phw = psB.tile([128, GR * HID], f32, tag="phw")
                    hidw = hidp.tile([128, GR * HID], bf16, tag="hidw")
                    grp = []
                    for r in range(GR):
                        t = g * GR + r
                        b, s = divmod(t, S)
                        if t % NCH == 0:
                            if t not in chunk_tiles:
                                issue_chunk(t)
                            s1c, s2c = chunk_tiles.pop(t)
                        col = (t % NCH) * 128
                        oh = ohp.tile([128, BLK], bf16, tag="oh")
                        nc.vector.tensor_scalar(
                            out=oh[:], in0=iota[:],
                            scalar1=rball[:, t:t + 1], scalar2=None,
                            op0=EQ)
                        ph = phw[:, r * HID:(r + 1) * HID]
                        nc.tensor.matmul(out=ph,
                                         lhsT=s1c[:, col:col + 128],
                                         rhs=wa[:], start=True, stop=False)
                        nc.tensor.matmul(out=ph,
                                         lhsT=s2c[:, col:col + 128],
                                         rhs=wb[:], start=False, stop=True)
                        grp.append((hidw, r, oh, b, s))
                    nc.scalar.activation(out=hidw[:], in_=phw[:], func=Relu)
                    pend.extend(grp)
                    if len(pend) > GR:
                        for _ in range(GR):
                            issue_agg(*pend.pop(0))
                while pend:
                    issue_agg(*pend.pop(0))
                while pend_c:
                    w_, hg_, _ = pend_c.pop(0)
                    phase_c(w_, hg_)

    nc.compile()
    return nc


def _balance_blocks(deg):
    """LPT bin-packing: nodes -> NBLK_ALL blocks of <=BLK nodes, balancing the
    per-block edge (receiver) totals."""
    import heapq
    order = np.argsort(-deg, kind="stable")
    heap = [(0, 0, b) for b in range(NBLK_ALL)]
    heapq.heapify(heap)
    block_of = np.empty(N_NODES, dtype=np.int32)
    slot_of = np.empty(N_NODES, dtype=np.int32)
    deg_l = deg.tolist()
    maxload = 0
    for n in order.tolist():
        load, cnt, b = heapq.heappop(heap)
        block_of[n] = b
        slot_of[n] = cnt
        load += deg_l[n]
        cnt += 1
        if load > maxload:
            maxload = load
        if cnt < BLK:
            heapq.heappush(heap, (load, cnt, b))
    return block_of, slot_of, maxload


def _host_prep(inputs):
    nf = np.ascontiguousarray(np.asarray(inputs["node_feat"], dtype=np.float32))
    ef = np.ascontiguousarray(np.asarray(inputs["edge_feat"], dtype=np.float32))
    snd = np.asarray(inputs["senders"]).astype(np.int64)
    rcv = np.asarray(inputs["receivers"]).astype(np.int64)
    We1 = np.asarray(inputs["We1"], dtype=np.float32)
    be1 = np.asarray(inputs["be1"], dtype=np.float32)
    We2 = np.asarray(inputs["We2"], dtype=np.float32)
    be2 = np.asarray(inputs["be2"], dtype=np.float32)
    Wn1 = np.asarray(inputs["Wn1"], dtype=np.float32)
    bn1 = np.asarray(inputs["bn1"], dtype=np.float32)
    Wn2 = np.asarray(inputs["Wn2"], dtype=np.float32)
    bn2 = np.asarray(inputs["bn2"], dtype=np.float32)

    deg_full = np.bincount(rcv, minlength=N_NODES).astype(np.int64)
    block_of, slot_of, maxload = _balance_blocks(deg_full)
    S = max(1, int(np.ceil(maxload / 128.0)))
    T = NBLK * S
    EPAD = T * 128

    core_of = (block_of // NBLK).astype(np.int32)       # node -> core
    blk_loc = (block_of % NBLK).astype(np.int64)        # node -> block in core
    pos_of = blk_loc * BLK + slot_of                    # node -> slot in core

    # per-edge routing (by receiver)
    e_core = core_of[rcv]
    e_blk = blk_loc[rcv]
    e_rb = slot_of[rcv].astype(np.float32)

    bf = BF
    wa = np.concatenate([We1[0:64], We1[128:192]], axis=0).astype(bf)
    wb = np.concatenate([We1[64:128], be1[None, :]], axis=0).astype(bf)
    wh1 = np.ascontiguousarray(We2 @ Wn1[:64]).astype(bf)
    wn1n = np.ascontiguousarray(Wn1[64:128]).astype(bf)
    c1 = np.ascontiguousarray((be2 @ Wn1[:64])[None, :]).astype(bf)
    bn1c = np.ascontiguousarray(bn1[:, None]).astype(np.float32)
    wn2 = np.ascontiguousarray(Wn2).astype(bf)
    bn2c = np.ascontiguousarray(bn2[:, None]).astype(np.float32)
    iota = np.ascontiguousarray(
        np.broadcast_to(np.arange(BLK, dtype=np.float32)[None, :], (128, BLK))
    ).astype(bf)

    in_maps = []
    for c in range(CORES):
        sel = np.nonzero(e_core == c)[0]
        blk = e_blk[sel]
        order = np.argsort(blk, kind="stable")
        sel = sel[order]
        blk = blk[order]
        cnts = np.bincount(blk, minlength=NBLK)
        starts = np.zeros(NBLK, dtype=np.int64)
        starts[1:] = np.cumsum(cnts)[:-1]
        within = np.arange(sel.size, dtype=np.int64) - starts[blk]
        col = blk * (S * 128) + within

        s1 = np.zeros((128, EPAD), dtype=bf)
        s1[0:64, col] = ef[sel].T
        s1[64:128, col] = nf[snd[sel]].T
        s2 = np.zeros((65, EPAD), dtype=bf)
        s2[0:64, col] = nf[rcv[sel]].T
        s2[64, col] = 1.0

        rbv = np.full((EPAD,), PAD_RB, dtype=np.float32)
        rbv[col] = e_rb[sel]
        rb_t = np.ascontiguousarray(rbv.reshape(T, 128).T)

        mine = np.nonzero(core_of == c)[0]
        nfloc = np.zeros((64, NLOC_PAD), dtype=bf)
        nfloc[:, pos_of[mine]] = nf[mine].T
        degl = np.zeros((1, NLOC_PAD), dtype=bf)
        degl[0, pos_of[mine]] = deg_full[mine].astype(bf)

        in_maps.append({
            "s1": s1, "s2": s2, "rb": rb_t, "nfloc": nfloc, "deg": degl,
            "wa": wa, "wb": wb, "wh1": wh1, "wn1n": wn1n, "c1": c1,
            "bn1c": bn1c, "wn2": wn2, "bn2c": bn2c, "iota": iota,
        })

    gpos = core_of.astype(np.int64) * NLOC_PAD + pos_of
    return S, in_maps, gpos


def _run(inputs, trace=False):
    from concourse.bass_utils import run_bass_kernel_spmd

    S, in_maps, gpos = _host_prep(inputs)
    if S not in _prog_cache:
        _prog_cache[S] = _build(S)
    nc = _prog_cache[S]
    res = run_bass_kernel_spmd(nc, in_maps, core_ids=list(range(CORES)),
                               trace=trace)
    big = np.concatenate(
        [np.asarray(res.results[c]["out_t"]) for c in range(CORES)], axis=1)
    out = np.ascontiguousarray(big[:, gpos].T.astype(np.float32))
    return out, res


def kernel(**inputs):
    out, _ = _run(inputs, trace=False)
    return out
